# revision 5
# baseline (speedup 1.0000x reference)
"""AdaPT_Linear (per-tensor int8-quantized linear) on 8 trn2 NeuronCores.

Strategy (data-parallel over rows of x):
  - Host passes x.T shards [1024, 2048] and w.T [1024, 1024] (replicated),
    so SBUF loads land with the contraction (IN) axis on partitions and no
    on-device transposes are needed.
  - Quantized int8 values are exact in bf16; int8xint8 products accumulate
    exactly in fp32 PSUM (max |acc| = 127*127*1024 < 2^24), so a bf16
    matmul reproduces the reference int32 matmul bit-exactly.
  - Per-tensor abs-max of x -> AllReduce(max) across the 8 cores; w/bias
    are replicated so their scales are computed locally.
  - round-half-even matches jnp.round via the +/-1.5*2^23 magic constant.
"""
import numpy as np

import concourse.bacc as bacc
import concourse.mybir as mybir
import concourse.tile as tile
from concourse import bass_isa
from concourse.bass_utils import run_bass_kernel_spmd

N_CORES = 8
N_ROWS = 16384
SIZE_IN = 1024
SIZE_OUT = 1024
ROWS_PER_CORE = N_ROWS // N_CORES          # 2048
K_TILES = SIZE_IN // 128                   # 8
R_TILES = ROWS_PER_CORE // 128             # 16
N_CHUNKS = SIZE_OUT // 512                 # 2
MAGIC = 12582912.0                         # 1.5 * 2**23: round-half-even trick
MAXV = 127.0

F32 = mybir.dt.float32
BF16 = mybir.dt.bfloat16


def build_nc():
    nc = bacc.Bacc(None, target_bir_lowering=False, debug=False,
                   num_devices=N_CORES)

    xt_ext = nc.declare_dram_parameter("xt", [SIZE_IN, ROWS_PER_CORE], F32,
                                       isOutput=False)
    wt_ext = nc.declare_dram_parameter("wt", [SIZE_IN, SIZE_OUT], F32,
                                       isOutput=False)
    b_ext = nc.declare_dram_parameter("bias", [1, SIZE_OUT], F32,
                                      isOutput=False)
    out_ext = nc.declare_dram_parameter("out", [ROWS_PER_CORE, SIZE_OUT], F32,
                                        isOutput=True)

    with tile.TileContext(nc) as tc:
        with (
            tc.tile_pool(name="big", bufs=1) as big,
            tc.tile_pool(name="stats", bufs=1) as stats,
            tc.tile_pool(name="ostage", bufs=4) as ostage,
            tc.tile_pool(name="psum", bufs=8, space="PSUM") as psum_pool,
            tc.tile_pool(name="dram", bufs=1, space="DRAM") as dram,
        ):
            xt_sb = [big.tile([128, ROWS_PER_CORE], F32, tag=f"xt{k}", name=f"xt{k}")
                     for k in range(K_TILES)]
            qxt_sb = [big.tile([128, ROWS_PER_CORE], BF16, tag=f"qxt{k}", name=f"qxt{k}")
                      for k in range(K_TILES)]
            wt_sb = [big.tile([128, SIZE_OUT], F32, tag=f"wt{k}", name=f"wt{k}")
                     for k in range(K_TILES)]
            qwt_sb = [big.tile([128, SIZE_OUT], BF16, tag=f"qwt{k}", name=f"qwt{k}")
                      for k in range(K_TILES)]

            xslots = stats.tile([128, K_TILES], F32, tag="xslots")
            wslots = stats.tile([128, K_TILES], F32, tag="wslots")
            xmax_l = stats.tile([128, 1], F32, tag="xmax_l")
            xmax_ar = stats.tile([128, 1], F32, tag="xmax_ar")
            xmax_g = stats.tile([128, 1], F32, tag="xmax_g")
            wmax_l = stats.tile([128, 1], F32, tag="wmax_l")
            wmax_g = stats.tile([128, 1], F32, tag="wmax_g")
            rx = stats.tile([128, 1], F32, tag="rx")
            rw = stats.tile([128, 1], F32, tag="rw")
            sa = stats.tile([128, 1], F32, tag="sa")
            sw = stats.tile([128, 1], F32, tag="sw")
            sd_t = stats.tile([128, 1], F32, tag="sd")
            b_sb = stats.tile([1, SIZE_OUT], F32, tag="b_sb")
            bmax = stats.tile([1, 1], F32, tag="bmax")
            rb = stats.tile([1, 1], F32, tag="rb")
            sb_t = stats.tile([1, 1], F32, tag="sb")
            bq = stats.tile([1, SIZE_OUT], F32, tag="bq")
            bval = stats.tile([1, SIZE_OUT], F32, tag="bval")
            bias_full = stats.tile([128, SIZE_OUT], F32, tag="bias_full")

            magic_c = stats.tile([128, 1], F32, tag="magic_c")
            nc.vector.memset(magic_c[:], MAGIC)

            cc_in = dram.tile([128, 1], F32, tag="cc_in")
            cc_out = dram.tile([128, 1], F32, tag="cc_out")

            # ---- load x shard (transposed layout) + per-tile |max| ----
            for k in range(K_TILES):
                nc.sync.dma_start(xt_sb[k][:], xt_ext[k * 128:(k + 1) * 128, :])
                nc.vector.tensor_reduce(
                    xslots[:, k:k + 1], xt_sb[k][:], axis=mybir.AxisListType.X,
                    op=mybir.AluOpType.max, apply_absolute_value=True)
            nc.vector.tensor_reduce(
                xmax_l[:], xslots[:], axis=mybir.AxisListType.X,
                op=mybir.AluOpType.max)

            # ---- global abs-max of x: AllReduce(max) over the 8 cores ----
            nc.gpsimd.dma_start(cc_in[:], xmax_l[:])
            nc.gpsimd.collective_compute(
                "AllReduce", mybir.AluOpType.max,
                replica_groups=[list(range(N_CORES))],
                ins=[cc_in.opt()], outs=[cc_out.opt()])
            nc.gpsimd.dma_start(xmax_ar[:], cc_out[:])
            nc.gpsimd.partition_all_reduce(
                xmax_g[:], xmax_ar[:], channels=128,
                reduce_op=bass_isa.ReduceOp.max)
            nc.vector.reciprocal(rx[:], xmax_g[:])
            nc.vector.tensor_scalar_mul(sa[:], rx[:], MAXV)

            # ---- weight load + local abs-max (replicated -> no collective) --
            for k in range(K_TILES):
                nc.sync.dma_start(wt_sb[k][:], wt_ext[k * 128:(k + 1) * 128, :])
                nc.vector.tensor_reduce(
                    wslots[:, k:k + 1], wt_sb[k][:], axis=mybir.AxisListType.X,
                    op=mybir.AluOpType.max, apply_absolute_value=True)
            nc.vector.tensor_reduce(
                wmax_l[:], wslots[:], axis=mybir.AxisListType.X,
                op=mybir.AluOpType.max)
            nc.gpsimd.partition_all_reduce(
                wmax_g[:], wmax_l[:], channels=128,
                reduce_op=bass_isa.ReduceOp.max)
            nc.vector.reciprocal(rw[:], wmax_g[:])
            nc.vector.tensor_scalar_mul(sw[:], rw[:], MAXV)

            # dequant scale: 1/(sa*sw) = xmax*wmax/127^2
            nc.vector.tensor_tensor(
                sd_t[:], xmax_g[:], wmax_g[:], op=mybir.AluOpType.mult)
            nc.vector.tensor_scalar_mul(sd_t[:], sd_t[:], 1.0 / (MAXV * MAXV))

            # ---- bias: quantize + dequantize locally, broadcast row ----
            nc.sync.dma_start(b_sb[:], b_ext[:])
            nc.vector.tensor_reduce(
                bmax[:], b_sb[:], axis=mybir.AxisListType.X,
                op=mybir.AluOpType.max, apply_absolute_value=True)
            nc.vector.reciprocal(rb[:], bmax[:])
            nc.vector.tensor_scalar_mul(sb_t[:], rb[:], MAXV)
            nc.vector.tensor_scalar(
                bq[:], b_sb[:], sb_t[:], MAGIC,
                op0=mybir.AluOpType.mult, op1=mybir.AluOpType.add)
            nc.vector.tensor_scalar(
                bq[:], bq[:], -MAGIC, None, op0=mybir.AluOpType.add,
                op1=mybir.AluOpType.bypass)
            # bias value row = qb/sb = qb * bmax / 127
            nc.vector.tensor_scalar(
                bval[:], bq[:], bmax[:], 1.0 / MAXV,
                op0=mybir.AluOpType.mult, op1=mybir.AluOpType.mult)
            nc.gpsimd.partition_broadcast(bias_full[:], bval[:], channels=128)

            # ---- quantize x and w to bf16 (round-half-even, no clip needed
            #      because the scale uses the global abs-max) ----
            for k in range(K_TILES):
                nc.scalar.activation(
                    xt_sb[k][:], xt_sb[k][:],
                    mybir.ActivationFunctionType.Identity,
                    bias=magic_c[:], scale=sa[:])
                nc.vector.tensor_scalar(
                    qxt_sb[k][:], xt_sb[k][:], -MAGIC, None,
                    op0=mybir.AluOpType.add, op1=mybir.AluOpType.bypass)
                nc.scalar.activation(
                    wt_sb[k][:], wt_sb[k][:],
                    mybir.ActivationFunctionType.Identity,
                    bias=magic_c[:], scale=sw[:])
                nc.vector.tensor_scalar(
                    qwt_sb[k][:], wt_sb[k][:], -MAGIC, None,
                    op0=mybir.AluOpType.add, op1=mybir.AluOpType.bypass)

            # ---- matmul + fused dequant/bias epilogue ----
            for r in range(R_TILES):
                for n in range(N_CHUNKS):
                    ps = psum_pool.tile([128, 512], F32, tag="ps")
                    for k in range(K_TILES):
                        nc.tensor.matmul(
                            ps[:],
                            qxt_sb[k][:, r * 128:(r + 1) * 128],
                            qwt_sb[k][:, n * 512:(n + 1) * 512],
                            start=(k == 0), stop=(k == K_TILES - 1))
                    ot = ostage.tile([128, 512], F32, tag="ot")
                    nc.vector.scalar_tensor_tensor(
                        ot[:], ps[:], sd_t[:],
                        bias_full[:, n * 512:(n + 1) * 512],
                        op0=mybir.AluOpType.mult, op1=mybir.AluOpType.add)
                    nc.sync.dma_start(
                        out_ext[r * 128:(r + 1) * 128,
                                n * 512:(n + 1) * 512], ot[:])

    nc.finalize()
    return nc


_NC_CACHE = None


def _get_nc():
    global _NC_CACHE
    if _NC_CACHE is None:
        _NC_CACHE = build_nc()
    return _NC_CACHE


def kernel(x, weight, bias):
    assert x.shape == (N_ROWS, SIZE_IN) and x.dtype == np.float32
    nc = _get_nc()
    wt = np.ascontiguousarray(weight.T)
    b2 = np.ascontiguousarray(bias.reshape(1, SIZE_OUT))
    in_maps = []
    for c in range(N_CORES):
        shard = np.ascontiguousarray(
            x[c * ROWS_PER_CORE:(c + 1) * ROWS_PER_CORE, :].T)
        in_maps.append({"xt": shard, "wt": wt, "bias": b2})
    res = run_bass_kernel_spmd(nc, in_maps, core_ids=list(range(N_CORES)))
    return np.concatenate([res.results[c]["out"] for c in range(N_CORES)],
                          axis=0)


# revision 6
# speedup vs baseline: 1.0022x; 1.0022x over previous
"""AdaPT_Linear (per-tensor int8-quantized linear) on 8 trn2 NeuronCores.

Strategy (data-parallel over rows of x):
  - Host passes x.T shards [1024, 2048] and w.T [1024, 1024] (replicated),
    so SBUF loads land with the contraction (IN) axis on partitions and no
    on-device transposes are needed.
  - Quantized int8 values are exact in bf16; int8xint8 products accumulate
    exactly in fp32 PSUM (max |acc| = 127*127*1024 < 2^24), so a bf16
    matmul reproduces the reference int32 matmul bit-exactly.
  - Per-tensor abs-max of x -> AllReduce(max) across the 8 cores; w/bias
    are replicated so their scales are computed locally.
  - round-half-even matches jnp.round via the +/-1.5*2^23 magic constant.
"""
import numpy as np

import concourse.bacc as bacc
import concourse.mybir as mybir
import concourse.tile as tile
from concourse import bass_isa
from concourse import library_config
from concourse.bass_utils import run_bass_kernel_spmd

N_CORES = 8
N_ROWS = 16384
SIZE_IN = 1024
SIZE_OUT = 1024
ROWS_PER_CORE = N_ROWS // N_CORES          # 2048
K_TILES = SIZE_IN // 128                   # 8
R_TILES = ROWS_PER_CORE // 128             # 16
N_CHUNKS = SIZE_OUT // 512                 # 2
MAGIC = 12582912.0                         # 1.5 * 2**23: round-half-even trick
MAXV = 127.0

F32 = mybir.dt.float32
BF16 = mybir.dt.bfloat16


def build_nc():
    nc = bacc.Bacc(None, target_bir_lowering=False, debug=False,
                   num_devices=N_CORES)

    xt_ext = nc.declare_dram_parameter("xt", [SIZE_IN, ROWS_PER_CORE], F32,
                                       isOutput=False)
    wt_ext = nc.declare_dram_parameter("wt", [SIZE_IN, SIZE_OUT], F32,
                                       isOutput=False)
    b_ext = nc.declare_dram_parameter("bias", [1, SIZE_OUT], F32,
                                      isOutput=False)
    out_ext = nc.declare_dram_parameter("out", [ROWS_PER_CORE, SIZE_OUT], F32,
                                        isOutput=True)

    with tile.TileContext(nc) as tc:
        with (
            tc.tile_pool(name="big", bufs=1) as big,
            tc.tile_pool(name="stats", bufs=1) as stats,
            tc.tile_pool(name="ostage", bufs=4) as ostage,
            tc.tile_pool(name="psum", bufs=8, space="PSUM") as psum_pool,
            tc.tile_pool(name="dram", bufs=1, space="DRAM") as dram,
        ):
            xt_sb = [big.tile([128, ROWS_PER_CORE], F32, tag=f"xt{k}", name=f"xt{k}")
                     for k in range(K_TILES)]
            qxt_sb = [big.tile([128, ROWS_PER_CORE], BF16, tag=f"qxt{k}", name=f"qxt{k}")
                      for k in range(K_TILES)]
            wt_sb = [big.tile([128, SIZE_OUT], F32, tag=f"wt{k}", name=f"wt{k}")
                     for k in range(K_TILES)]
            qwt_sb = [big.tile([128, SIZE_OUT], BF16, tag=f"qwt{k}", name=f"qwt{k}")
                      for k in range(K_TILES)]

            xslots = stats.tile([128, K_TILES], F32, tag="xslots")
            wslots = stats.tile([128, K_TILES], F32, tag="wslots")
            xmax_l = stats.tile([128, 1], F32, tag="xmax_l")
            xmax_ar = stats.tile([128, 1], F32, tag="xmax_ar")
            xmax_g = stats.tile([128, 1], F32, tag="xmax_g")
            wmax_l = stats.tile([128, 1], F32, tag="wmax_l")
            wmax_g = stats.tile([128, 1], F32, tag="wmax_g")
            rx = stats.tile([128, 1], F32, tag="rx")
            rw = stats.tile([128, 1], F32, tag="rw")
            sa = stats.tile([128, 1], F32, tag="sa")
            sw = stats.tile([128, 1], F32, tag="sw")
            sd_t = stats.tile([128, 1], F32, tag="sd")
            b_sb = stats.tile([1, SIZE_OUT], F32, tag="b_sb")
            bmax = stats.tile([1, 1], F32, tag="bmax")
            rb = stats.tile([1, 1], F32, tag="rb")
            sb_t = stats.tile([1, 1], F32, tag="sb")
            bq = stats.tile([1, SIZE_OUT], F32, tag="bq")
            bval = stats.tile([1, SIZE_OUT], F32, tag="bval")
            bias_full = stats.tile([128, SIZE_OUT], F32, tag="bias_full")

            magic_c = stats.tile([128, 1], F32, tag="magic_c")
            nc.vector.memset(magic_c[:], MAGIC)
            # preload the gpsimd library needed by partition_all_reduce /
            # partition_broadcast so the ~14us ucode reload overlaps the x
            # load instead of stalling the abs-max collective chain
            nc.gpsimd.load_library(library_config.attn)

            cc_in = dram.tile([128, 1], F32, tag="cc_in")
            cc_out = dram.tile([128, 1], F32, tag="cc_out")

            # ---- load x shard (transposed layout) + per-tile |max| ----
            for k in range(K_TILES):
                nc.sync.dma_start(xt_sb[k][:], xt_ext[k * 128:(k + 1) * 128, :])
                nc.vector.tensor_reduce(
                    xslots[:, k:k + 1], xt_sb[k][:], axis=mybir.AxisListType.X,
                    op=mybir.AluOpType.max, apply_absolute_value=True)
            nc.vector.tensor_reduce(
                xmax_l[:], xslots[:], axis=mybir.AxisListType.X,
                op=mybir.AluOpType.max)

            # ---- global abs-max of x: AllReduce(max) over the 8 cores ----
            nc.gpsimd.dma_start(cc_in[:], xmax_l[:])
            nc.gpsimd.collective_compute(
                "AllReduce", mybir.AluOpType.max,
                replica_groups=[list(range(N_CORES))],
                ins=[cc_in.opt()], outs=[cc_out.opt()])
            nc.gpsimd.dma_start(xmax_ar[:], cc_out[:])
            nc.gpsimd.partition_all_reduce(
                xmax_g[:], xmax_ar[:], channels=128,
                reduce_op=bass_isa.ReduceOp.max)
            nc.vector.reciprocal(rx[:], xmax_g[:])
            nc.vector.tensor_scalar_mul(sa[:], rx[:], MAXV)

            # ---- weight load + local abs-max (replicated -> no collective) --
            for k in range(K_TILES):
                nc.sync.dma_start(wt_sb[k][:], wt_ext[k * 128:(k + 1) * 128, :])
                nc.vector.tensor_reduce(
                    wslots[:, k:k + 1], wt_sb[k][:], axis=mybir.AxisListType.X,
                    op=mybir.AluOpType.max, apply_absolute_value=True)
            nc.vector.tensor_reduce(
                wmax_l[:], wslots[:], axis=mybir.AxisListType.X,
                op=mybir.AluOpType.max)
            nc.gpsimd.partition_all_reduce(
                wmax_g[:], wmax_l[:], channels=128,
                reduce_op=bass_isa.ReduceOp.max)
            nc.vector.reciprocal(rw[:], wmax_g[:])
            nc.vector.tensor_scalar_mul(sw[:], rw[:], MAXV)

            # dequant scale: 1/(sa*sw) = xmax*wmax/127^2
            nc.vector.tensor_tensor(
                sd_t[:], xmax_g[:], wmax_g[:], op=mybir.AluOpType.mult)
            nc.vector.tensor_scalar_mul(sd_t[:], sd_t[:], 1.0 / (MAXV * MAXV))

            # ---- bias: quantize + dequantize locally, broadcast row ----
            nc.sync.dma_start(b_sb[:], b_ext[:])
            nc.vector.tensor_reduce(
                bmax[:], b_sb[:], axis=mybir.AxisListType.X,
                op=mybir.AluOpType.max, apply_absolute_value=True)
            nc.vector.reciprocal(rb[:], bmax[:])
            nc.vector.tensor_scalar_mul(sb_t[:], rb[:], MAXV)
            nc.vector.tensor_scalar(
                bq[:], b_sb[:], sb_t[:], MAGIC,
                op0=mybir.AluOpType.mult, op1=mybir.AluOpType.add)
            nc.vector.tensor_scalar(
                bq[:], bq[:], -MAGIC, None, op0=mybir.AluOpType.add,
                op1=mybir.AluOpType.bypass)
            # bias value row = qb/sb = qb * bmax / 127
            nc.vector.tensor_scalar(
                bval[:], bq[:], bmax[:], 1.0 / MAXV,
                op0=mybir.AluOpType.mult, op1=mybir.AluOpType.mult)
            nc.gpsimd.partition_broadcast(bias_full[:], bval[:], channels=128)

            # ---- quantize x and w to bf16 (round-half-even, no clip needed
            #      because the scale uses the global abs-max) ----
            for k in range(K_TILES):
                nc.scalar.activation(
                    xt_sb[k][:], xt_sb[k][:],
                    mybir.ActivationFunctionType.Identity,
                    bias=magic_c[:], scale=sa[:])
                nc.vector.tensor_scalar(
                    qxt_sb[k][:], xt_sb[k][:], -MAGIC, None,
                    op0=mybir.AluOpType.add, op1=mybir.AluOpType.bypass)
                nc.scalar.activation(
                    wt_sb[k][:], wt_sb[k][:],
                    mybir.ActivationFunctionType.Identity,
                    bias=magic_c[:], scale=sw[:])
                nc.vector.tensor_scalar(
                    qwt_sb[k][:], wt_sb[k][:], -MAGIC, None,
                    op0=mybir.AluOpType.add, op1=mybir.AluOpType.bypass)

            # ---- matmul + fused dequant/bias epilogue ----
            for r in range(R_TILES):
                for n in range(N_CHUNKS):
                    ps = psum_pool.tile([128, 512], F32, tag="ps")
                    for k in range(K_TILES):
                        nc.tensor.matmul(
                            ps[:],
                            qxt_sb[k][:, r * 128:(r + 1) * 128],
                            qwt_sb[k][:, n * 512:(n + 1) * 512],
                            start=(k == 0), stop=(k == K_TILES - 1))
                    ot = ostage.tile([128, 512], F32, tag="ot")
                    nc.vector.scalar_tensor_tensor(
                        ot[:], ps[:], sd_t[:],
                        bias_full[:, n * 512:(n + 1) * 512],
                        op0=mybir.AluOpType.mult, op1=mybir.AluOpType.add)
                    nc.sync.dma_start(
                        out_ext[r * 128:(r + 1) * 128,
                                n * 512:(n + 1) * 512], ot[:])

    nc.finalize()
    return nc


_NC_CACHE = None


def _get_nc():
    global _NC_CACHE
    if _NC_CACHE is None:
        _NC_CACHE = build_nc()
    return _NC_CACHE


def kernel(x, weight, bias):
    assert x.shape == (N_ROWS, SIZE_IN) and x.dtype == np.float32
    nc = _get_nc()
    wt = np.ascontiguousarray(weight.T)
    b2 = np.ascontiguousarray(bias.reshape(1, SIZE_OUT))
    in_maps = []
    for c in range(N_CORES):
        shard = np.ascontiguousarray(
            x[c * ROWS_PER_CORE:(c + 1) * ROWS_PER_CORE, :].T)
        in_maps.append({"xt": shard, "wt": wt, "bias": b2})
    res = run_bass_kernel_spmd(nc, in_maps, core_ids=list(range(N_CORES)))
    return np.concatenate([res.results[c]["out"] for c in range(N_CORES)],
                          axis=0)
